# revision 30
# baseline (speedup 1.0000x reference)
"""Trainium2 Bass kernel for GQA attention (nn_Attention_36739150250506).

B=1, S=2048, H=2048, NH=16 q-heads, NKV=4 kv-heads, D=128.
Sharding: 2 q-heads + their (shared) kv-head per core; o-proj partial sums
reduced on host.

Single pipelined loop over 512-wide sequence panels keeps the tensor engine
dense (HAM stays warm). Per panel p (all matmuls contract over partitions):
  1. qkv = xT.T @ [wq0|wq1|wk|wv] for the panel's four 128-row s-tiles;
     RMSNorm+RoPE in [s,d] layout (norm weights folded into host cos/sin
     tables); PE-transpose q/k to [d,s]; v kept [s,d] bf16.
  2. scoresT[k, q-chunk p] = kT_i.T @ qT for all causal k-tiles i; diag
     mask add; exp on ACT (scale=1/sqrt(D)) into per-column bf16 buffers.
  3. attn@v for q-chunk p (v.T @ expT, unnormalized) + denominator via
     ones-matmul; denominator rows PE-transposed to per-partition layout,
     reciprocal into a [s-tile] table.
  4. o-proj for q-chunk p-1: per-head matmuls, evict applies the
     reciprocal as a per-partition scale, partial written to HBM in bf16.
"""

import math

import numpy as np

S = 2048
H = 2048
D = 128
NH = 16
NKV = 4
HQ = 2  # q heads per core
N_CORES = 8
EPS = 1.1920928955078125e-07
NEG = -1e30
CFG = ("bf16", "bf16", "bf16")  # (dt_x, dt_attn, dt_oproj)
OUT_BF16 = True


def build_nc(s=S, h=H, dt_x=None, dt_attn=None, dt_oproj=None):
    dt_x = dt_x or CFG[0]
    dt_attn = dt_attn or CFG[1]
    dt_oproj = dt_oproj or CFG[2]
    import concourse.bacc as bacc
    import concourse.mybir as mybir
    import concourse.tile as tile

    f32 = mybir.dt.float32
    f32r = mybir.dt.float32r
    bf16 = mybir.dt.bfloat16
    Act = mybir.ActivationFunctionType
    Alu = mybir.AluOpType

    def dtt(name):
        return {"f32": f32, "f32r": f32r, "bf16": bf16}[name]

    x_st = dtt(dt_x)
    at_st = dtt(dt_attn)
    op_st = dtt(dt_oproj)

    ST = s // 128   # 128-row s-tiles
    HT = h // 128   # 128-row h-tiles (contraction tiles)
    NOC = h // 512  # 512-wide outdim chunks
    NP = s // 512   # 512-wide panels / q-chunks

    nc = bacc.Bacc("TRN2", target_bir_lowering=False, debug=False,
                   num_devices=N_CORES)

    xT = nc.dram_tensor("xT", [h, s], x_st, kind="ExternalInput")
    wall = nc.dram_tensor("wall", [h, 512], x_st, kind="ExternalInput")
    woT = nc.dram_tensor("woT", [HQ * 128, h], op_st, kind="ExternalInput")
    csn = nc.dram_tensor("csn", [s, 512], f32, kind="ExternalInput")
    cmask = nc.dram_tensor("cmask", [128, 128], f32, kind="ExternalInput")
    ident = nc.dram_tensor("ident", [128, 128], f32, kind="ExternalInput")
    out_dt = bf16 if OUT_BF16 else f32
    out = nc.dram_tensor("out", [s, h], out_dt, kind="ExternalOutput")

    inv_sqrt_d = 1.0 / math.sqrt(D)

    with tile.TileContext(nc) as tc:
        with tc.tile_pool(name="resident", bufs=1) as res_pool, \
             tc.tile_pool(name="ecol", bufs=1) as ec_pool, \
             tc.tile_pool(name="xpanel", bufs=2) as xp_pool, \
             tc.tile_pool(name="csn_p", bufs=3) as csn_pool, \
             tc.tile_pool(name="ph1tmp", bufs=2) as tmp_pool, \
             tc.tile_pool(name="dn_sb_p", bufs=2) as dnsb_pool, \
             tc.tile_pool(name="out_sb", bufs=4) as out_pool, \
             tc.tile_pool(name="mm_ps", bufs=3, space="PSUM") as mm_psp, \
             tc.tile_pool(name="tr_ps", bufs=1, space="PSUM") as tr_psp, \
             tc.tile_pool(name="oop_ps", bufs=3, space="PSUM") as oop_psp, \
             tc.tile_pool(name="dn_ps", bufs=1, space="PSUM") as dn_psp:
            qkT_sb = res_pool.tile([128, 3 * s], f32r)
            v_sb = res_pool.tile([128, s], at_st)
            ones_sb = res_pool.tile([128, 1], at_st)
            ident_sb = res_pool.tile([128, 128], f32)
            cmask_sb = res_pool.tile([128, 128], f32)
            oT_sb = res_pool.tile([128, HQ * s], op_st)
            woT_sb = res_pool.tile([128, HQ * h], op_st)
            eps_sb = res_pool.tile([128, 1], f32)
            rcT_sb = res_pool.tile([128, HQ * ST], f32)
            wall_sb = res_pool.tile([128, HT * 512], x_st)
            nc.vector.memset(eps_sb[:], EPS)
            nc.vector.memset(ones_sb[:], 1.0)
            nc.sync.dma_start(ident_sb[:], ident[:, :])
            nc.sync.dma_start(cmask_sb[:], cmask[:, :])

            # exp column buffers: ec[hd][i] holds exp(scoresT) for k-tile i,
            # current q-chunk. Written in panel p, consumed by attn@v(p).
            ecs = [[ec_pool.tile([128, 512], at_st, tag=f"ec{hd}_{i}",
                                 name=f"ec{hd}_{i}")
                    for i in range(ST)] for hd in range(HQ)]

            wq4 = 2

            def load_wall_chunk(c):
                nc.sync.dma_start(
                    wall_sb[:, c * 512:(c + wq4) * 512]
                    .rearrange("p (t n) -> p t n", n=512),
                    wall[c * 128:(c + wq4) * 128, :]
                    .rearrange("(t p) n -> p t n", p=128))

            def emit_proj_mm(m, xp, mi, ssq_p):
                """projection matmuls + squares + raw copies for s-tile m;
                returns SBUF tiles for the deferred rope."""
                qkv_ps = mm_psp.tile([128, 512], f32, tag="mm", name="qkv_ps")
                for t in range(HT):
                    nc.tensor.matmul(
                        qkv_ps[:],
                        lhsT=xp[:, t * 512 + mi * 128:t * 512 + mi * 128 + 128],
                        rhs=wall_sb[:, t * 512:(t + 1) * 512],
                        start=(t == 0), stop=(t == HT - 1))

                cs = csn_pool.tile([128, 512], f32, tag="cs", name="cs",
                                   bufs=6)
                nc.sync.dma_start(cs[:], csn[m * 128:(m + 1) * 128, :])

                sq = tmp_pool.tile([128, 128], f32, tag="sq", name="sq")
                qraw = tmp_pool.tile([128, 384], f32, tag="qraw",
                                     name="qraw", bufs=5)
                rot = tmp_pool.tile([128, 384], f32, tag="rot", name="rot",
                                    bufs=5)
                for g in range(3):
                    nc.scalar.activation(
                        sq[:], qkv_ps[:, g * 128:(g + 1) * 128],
                        Act.Square, accum_out=ssq_p[:, mi * 3 + g:
                                                    mi * 3 + g + 1])
                nc.scalar.copy(qraw[:], qkv_ps[:, 0:384])
                src_h = qkv_ps[:, 0:384].rearrange(
                    "p (g two e) -> p g two e", two=2, e=64)
                rot_h = rot.rearrange("p (g two e) -> p g two e", two=2, e=64)
                nc.vector.tensor_copy(rot_h[:, :, 0, :], src_h[:, :, 1, :])
                nc.vector.tensor_copy(rot_h[:, :, 1, :], src_h[:, :, 0, :])
                nc.scalar.copy(v_sb[:, m * 128:(m + 1) * 128],
                               qkv_ps[:, 384:512])
                return qraw, rot, cs

            def emit_rsqrt(ssq_p):
                """rs = 1/sqrt(ssq/D + eps) via Newton (no ACT table swap).
                Inputs are unit-variance so v is within [0.5, 2]; five
                iterations from y0=1 converge below fp32 epsilon."""
                w = 12
                v = tmp_pool.tile([128, w], f32, tag="nv", name="v", bufs=2)
                y = tmp_pool.tile([128, w], f32, tag="ny", name="y", bufs=2)
                t1 = tmp_pool.tile([128, w], f32, tag="nt1", name="t1",
                                   bufs=2)
                r = tmp_pool.tile([128, w], f32, tag="nr", name="r", bufs=2)
                nc.vector.tensor_scalar(v[:], ssq_p[:, 0:w], 1.0 / D, EPS,
                                        op0=Alu.mult, op1=Alu.add)
                nc.vector.reciprocal(r[:], v[:])
                # rsqrt(r) by Newton; r <= ~1.3 so the linear seed always
                # converges. rs = v^-1/2 = r * rsqrt(r).
                nc.vector.tensor_scalar(y[:], r[:], -0.5, 1.5,
                                        op0=Alu.mult, op1=Alu.add)
                for _ in range(4):
                    nc.vector.tensor_mul(t1[:], y[:], y[:])
                    nc.vector.tensor_mul(t1[:], t1[:], r[:])
                    nc.vector.tensor_scalar(t1[:], t1[:], -0.5, 1.5,
                                            op0=Alu.mult, op1=Alu.add)
                    nc.vector.tensor_mul(y[:], y[:], t1[:])
                nc.vector.tensor_mul(y[:], y[:], r[:])
                return y

            def emit_rope(m, mi, qraw, rot, cs, rs):
                """deferred rope + transposes for s-tile m"""
                qn = tmp_pool.tile([128, 384], f32, tag="qn", name="qn")
                t2 = tmp_pool.tile([128, 384], f32, tag="t2", name="t2")
                for g in range(3):
                    c0 = 0 if g < 2 else 128
                    nc.vector.scalar_tensor_tensor(
                        out=qn[:, g * 128:(g + 1) * 128],
                        in0=qraw[:, g * 128:(g + 1) * 128],
                        scalar=rs[:, mi * 3 + g:mi * 3 + g + 1],
                        in1=cs[:, c0:c0 + 128],
                        op0=Alu.mult, op1=Alu.mult)
                    nc.vector.scalar_tensor_tensor(
                        out=t2[:, g * 128:(g + 1) * 128],
                        in0=rot[:, g * 128:(g + 1) * 128],
                        scalar=rs[:, mi * 3 + g:mi * 3 + g + 1],
                        in1=cs[:, 256 + c0:256 + c0 + 128],
                        op0=Alu.mult, op1=Alu.mult)
                nc.gpsimd.tensor_add(qn[:, 0:384], qn[:, 0:384],
                                     t2[:, 0:384])
                trp = tr_psp.tile([128, 384], f32, tag="trp", name="trp")
                for g in range(3):
                    nc.tensor.transpose(
                        trp[:, g * 128:(g + 1) * 128],
                        qn[:, g * 128:(g + 1) * 128], ident_sb[:])
                dst = qkT_sb.rearrange("p (g x) -> p g x", g=3)[
                    :, :, m * 128:m * 128 + 128]
                nc.scalar.copy(dst, trp.rearrange("p (g x) -> p g x", g=3))

            def emit_scores(i, pc):
                """scoresT chunk for k-tile i, q-chunk pc, both heads"""
                dlo = 128 * i
                seg = pc * 512
                lo = max(seg, dlo)
                for hd in range(HQ):
                    ec = ecs[hd][i]
                    if lo > seg:
                        nc.vector.memset(ec[:, 0:lo - seg], 0.0)
                    sc = mm_psp.tile([128, 512], f32, tag="mm", name="sc")
                    nc.tensor.matmul(
                        sc[:, lo - seg:512],
                        lhsT=qkT_sb[:, 2 * s + i * 128:2 * s + (i + 1) * 128],
                        rhs=qkT_sb[:, hd * s + lo:hd * s + seg + 512],
                        start=True, stop=True)
                    if seg <= dlo:
                        nc.vector.tensor_add(
                            sc[:, dlo - seg:dlo - seg + 128],
                            sc[:, dlo - seg:dlo - seg + 128],
                            cmask_sb[:])
                    nc.scalar.activation(
                        ec[:, lo - seg:512], sc[:, lo - seg:512],
                        Act.Exp, scale=inv_sqrt_d)

            def start_attnv(pc):
                dn_ps = dn_psp.tile([33, 512], f32, tag="dn", name="dn_ps")
                o_ps0 = oop_psp.tile([128, 512], f32, tag="oop", name="o_ps0")
                o_ps1 = oop_psp.tile([128, 512], f32, tag="oop", name="o_ps1")
                return (dn_ps, [o_ps0, o_ps1])

            def attnv_mm(i, pc, st):
                imax = min(ST, 4 * pc + 4)
                dn_ps, o_pss = st
                for hd in range(HQ):
                    nc.tensor.matmul(o_pss[hd][:],
                                     lhsT=v_sb[:, i * 128:(i + 1) * 128],
                                     rhs=ecs[hd][i][:],
                                     start=(i == 0), stop=(i == imax - 1))
                    nc.tensor.matmul(dn_ps[hd * 32:hd * 32 + 1, :],
                                     lhsT=ones_sb[:],
                                     rhs=ecs[hd][i][:],
                                     start=(i == 0), stop=(i == imax - 1))

            def finish_attnv(pc, st):
                dn_ps, o_pss = st
                for hd in range(HQ):
                    dst = oT_sb[:, hd * s + pc * 512:hd * s + (pc + 1) * 512]
                    if hd == 0:
                        nc.scalar.copy(dst, o_pss[hd][:])
                    else:
                        nc.vector.tensor_copy(dst, o_pss[hd][:])
                dn_sb = dnsb_pool.tile([33, 512], f32, tag="dn_sb",
                                       name="dn_sb")
                nc.scalar.copy(dn_sb[:], dn_ps[:])
                dnt = tr_psp.tile([128, HQ * 4], f32, tag="trp", name="dnt")
                for hd in range(HQ):
                    for c in range(4):
                        nc.tensor.transpose(
                            dnt[:, hd * 4 + c:hd * 4 + c + 1],
                            dn_sb[hd * 32:hd * 32 + 1,
                                  c * 128:(c + 1) * 128],
                            ident_sb[hd * 32:hd * 32 + 1,
                                     hd * 32:hd * 32 + 1])
                for hd in range(HQ):
                    nc.vector.reciprocal(
                        rcT_sb[:, hd * ST + pc * 4:hd * ST + pc * 4 + 4],
                        dnt[:, hd * 4:hd * 4 + 4])

            def emit_oproj(pc):
                """output projection for the s-tiles of q-chunk pc"""
                for m4 in range(4):
                    m = pc * 4 + m4
                    for n in range(NOC):
                        ops = []
                        for hd in range(HQ):
                            op_ps = oop_psp.tile([128, 512], f32, tag="oop",
                                                 name="op_ps")
                            ops.append(op_ps)
                            nc.tensor.matmul(
                                op_ps[:],
                                lhsT=oT_sb[:, hd * s + m * 128:
                                           hd * s + m * 128 + 128],
                                rhs=woT_sb[:, hd * h + n * 512:
                                           hd * h + (n + 1) * 512],
                                start=True, stop=True)
                        ob = out_pool.tile([128, 512], out_dt, tag="ob",
                                           name="ob")
                        r0 = rcT_sb[:, 0 * ST + m:0 * ST + m + 1]
                        r1 = rcT_sb[:, 1 * ST + m:1 * ST + m + 1]
                        if (m * NOC + n) % 2 == 0:
                            nc.scalar.activation(ob[:], ops[0][:], Act.Copy,
                                                 scale=r0)
                        else:
                            nc.vector.tensor_scalar_mul(ob[:], ops[0][:], r0)
                        nc.vector.scalar_tensor_tensor(
                            out=ob[:], in0=ops[1][:], scalar=r1,
                            in1=ob[:], op0=Alu.mult, op1=Alu.add)
                        nc.sync.dma_start(
                            out[m * 128:(m + 1) * 128, n * 512:(n + 1) * 512],
                            ob[:])

            # ------- pipelined panel loop: proj(p) | scores+attnv(p-1)
            # | o-proj(p-2) ----------------------------------------------
            for pn in range(NP + 1):
                if pn < NP:
                    xp = xp_pool.tile([128, HT * 512], x_st, tag="xp",
                                      name="xp")
                    xh = 2 if pn == 0 else max(1, HT // 4)
                    for c in range(0, HT, xh):
                        if pn == 0:
                            for cw in range(c, c + xh, wq4):
                                load_wall_chunk(cw)
                        nc.sync.dma_start(
                            xp[:, c * 512:(c + xh) * 512]
                            .rearrange("p (t e) -> p t e", e=512),
                            xT[c * 128:(c + xh) * 128,
                               pn * 512:(pn + 1) * 512]
                            .rearrange("(t p) e -> p t e", p=128))
                    if pn == min(1, NP - 1):
                        for g in range(HQ):
                            nc.sync.dma_start(
                                woT_sb[:, g * h:(g + 1) * h],
                                woT[g * 128:(g + 1) * 128, :]
                                .rearrange("(g p) o -> p (g o)", g=1))
                sc_list = (list(range(min(ST, 4 * (pn - 1) + 4)))
                           if pn >= 1 else [])
                parts = []
                if pn < NP:
                    ssq_p = tmp_pool.tile([128, 12], f32, tag="ssq_p",
                                          name="ssq_p", bufs=2)
                st = start_attnv(pn - 1) if pn >= 1 else None
                for mi in range(4):
                    if pn < NP:
                        parts.append(emit_proj_mm(pn * 4 + mi, xp, mi, ssq_p))
                    for i in sc_list[mi::4]:
                        emit_scores(i, pn - 1)
                        attnv_mm(i, pn - 1, st)
                if pn >= 1:
                    finish_attnv(pn - 1, st)
                if pn >= 2:
                    emit_oproj(pn - 2)
                if pn < NP:
                    rs = emit_rsqrt(ssq_p)
                    for mi in range(4):
                        qraw, rot, cs = parts[mi]
                        emit_rope(pn * 4 + mi, mi, qraw, rot, cs, rs)
            emit_oproj(NP - 1)

    nc.finalize()
    return nc


def host_inputs(x, sin, cos, wq, wk, wv, wo, q_norm_w, k_norm_w,
                dt_x=None, dt_oproj=None):
    """Build the 8 per-core input maps from full inputs."""
    dt_x = dt_x or CFG[0]
    dt_oproj = dt_oproj or CFG[2]
    import ml_dtypes
    bfnp = ml_dtypes.bfloat16

    def cast_x(a):
        if dt_x == "bf16":
            return np.ascontiguousarray(a).astype(bfnp)
        return np.ascontiguousarray(a, dtype=np.float32)

    def cast_o(a):
        if dt_oproj == "bf16":
            return np.ascontiguousarray(a).astype(bfnp)
        return np.ascontiguousarray(a, dtype=np.float32)

    xT = cast_x(x[0].T)

    sin2 = sin[0, 0]  # [S, D]
    cos2 = cos[0, 0]
    sign = np.where(np.arange(D) < 64, -1.0, 1.0).astype(np.float32)
    perm = (np.arange(D) + 64) % D
    cos_q = (cos2 * q_norm_w[None, :]).astype(np.float32)
    cos_k = (cos2 * k_norm_w[None, :]).astype(np.float32)
    sin_q = (sin2 * sign[None, :] * q_norm_w[None, perm]).astype(np.float32)
    sin_k = (sin2 * sign[None, :] * k_norm_w[None, perm]).astype(np.float32)
    csn = np.ascontiguousarray(
        np.concatenate([cos_q, cos_k, sin_q, sin_k], axis=1), dtype=np.float32)

    kq = np.arange(128)
    cmask = np.where(kq[None, :] >= kq[:, None], 0.0, NEG).astype(np.float32)
    ident = np.eye(128, dtype=np.float32)

    in_maps = []
    for d in range(N_CORES):
        kv = d // 2
        wall = cast_x(np.concatenate(
            [wq[256 * d:256 * (d + 1)],
             wk[128 * kv:128 * (kv + 1)],
             wv[128 * kv:128 * (kv + 1)]], axis=0).T)
        woTd = cast_o(wo[:, 256 * d:256 * (d + 1)].T)
        in_maps.append({"xT": xT, "wall": wall, "woT": woTd, "csn": csn,
                        "cmask": cmask, "ident": ident})
    return in_maps


_NC_CACHE = {}


def kernel(x, sin, cos, mask, wq, wk, wv, wo, q_norm_w, k_norm_w,
           _trace=False):
    from concourse.bass_utils import run_bass_kernel_spmd

    key = CFG
    if key not in _NC_CACHE:
        _NC_CACHE[key] = build_nc(s=S, h=H)
    nc = _NC_CACHE[key]

    in_maps = host_inputs(np.asarray(x), np.asarray(sin), np.asarray(cos),
                          np.asarray(wq), np.asarray(wk), np.asarray(wv),
                          np.asarray(wo), np.asarray(q_norm_w),
                          np.asarray(k_norm_w))
    res = run_bass_kernel_spmd(nc, in_maps, core_ids=list(range(N_CORES)),
                               trace=_trace)
    acc = np.zeros((S, H), dtype=np.float32)
    for d in range(N_CORES):
        acc += np.asarray(res.results[d]["out"], dtype=np.float32)
    outv = acc.reshape(1, S, H)
    if _trace:
        return outv, res
    return outv


# revision 31
# speedup vs baseline: 1.1117x; 1.1117x over previous
"""Trainium2 Bass kernel for GQA attention (nn_Attention_36739150250506).

B=1, S=2048, H=2048, NH=16 q-heads, NKV=4 kv-heads, D=128.
Sharding: 2 q-heads + their (shared) kv-head per core; o-proj partial sums
reduced on host.

Single pipelined loop over 512-wide sequence panels keeps the tensor engine
dense (HAM stays warm). Per panel p (all matmuls contract over partitions):
  1. qkv = xT.T @ [wq0|wq1|wk|wv] for the panel's four 128-row s-tiles;
     RMSNorm+RoPE in [s,d] layout (norm weights folded into host cos/sin
     tables); PE-transpose q/k to [d,s]; v kept [s,d] bf16.
  2. scoresT[k, q-chunk p] = kT_i.T @ qT for all causal k-tiles i; diag
     mask add; exp on ACT (scale=1/sqrt(D)) into per-column bf16 buffers.
  3. attn@v for q-chunk p (v.T @ expT, unnormalized) + denominator via
     ones-matmul; denominator rows PE-transposed to per-partition layout,
     reciprocal into a [s-tile] table.
  4. o-proj for q-chunk p-1: per-head matmuls, evict applies the
     reciprocal as a per-partition scale, partial written to HBM in bf16.
"""

import math

import numpy as np

S = 2048
H = 2048
D = 128
NH = 16
NKV = 4
HQ = 2  # q heads per core
N_CORES = 8
EPS = 1.1920928955078125e-07
NEG = -1e30
CFG = ("bf16", "bf16", "bf16")  # (dt_x, dt_attn, dt_oproj)
OUT_BF16 = True


def build_nc(s=S, h=H, dt_x=None, dt_attn=None, dt_oproj=None):
    dt_x = dt_x or CFG[0]
    dt_attn = dt_attn or CFG[1]
    dt_oproj = dt_oproj or CFG[2]
    import concourse.bacc as bacc
    import concourse.mybir as mybir
    import concourse.tile as tile

    f32 = mybir.dt.float32
    f32r = mybir.dt.float32r
    bf16 = mybir.dt.bfloat16
    Act = mybir.ActivationFunctionType
    Alu = mybir.AluOpType

    def dtt(name):
        return {"f32": f32, "f32r": f32r, "bf16": bf16}[name]

    x_st = dtt(dt_x)
    at_st = dtt(dt_attn)
    op_st = dtt(dt_oproj)

    ST = s // 128   # 128-row s-tiles
    HT = h // 128   # 128-row h-tiles (contraction tiles)
    NOC = h // 512  # 512-wide outdim chunks
    NP = s // 512   # 512-wide panels / q-chunks

    nc = bacc.Bacc("TRN2", target_bir_lowering=False, debug=False,
                   num_devices=N_CORES)

    xT = nc.dram_tensor("xT", [h, s], x_st, kind="ExternalInput")
    wall = nc.dram_tensor("wall", [h, 512], x_st, kind="ExternalInput")
    woT = nc.dram_tensor("woT", [HQ * 128, h], op_st, kind="ExternalInput")
    csn = nc.dram_tensor("csn", [s, 512], f32, kind="ExternalInput")
    cmask = nc.dram_tensor("cmask", [128, 128], f32, kind="ExternalInput")
    ident = nc.dram_tensor("ident", [128, 128], f32, kind="ExternalInput")
    out_dt = bf16 if OUT_BF16 else f32
    out = nc.dram_tensor("out", [s, h], out_dt, kind="ExternalOutput")

    inv_sqrt_d = 1.0 / math.sqrt(D)

    with tile.TileContext(nc) as tc:
        with tc.tile_pool(name="resident", bufs=1) as res_pool, \
             tc.tile_pool(name="ecol", bufs=1) as ec_pool, \
             tc.tile_pool(name="xpanel", bufs=2) as xp_pool, \
             tc.tile_pool(name="csn_p", bufs=3) as csn_pool, \
             tc.tile_pool(name="ph1tmp", bufs=2) as tmp_pool, \
             tc.tile_pool(name="dn_sb_p", bufs=2) as dnsb_pool, \
             tc.tile_pool(name="out_sb", bufs=4) as out_pool, \
             tc.tile_pool(name="mm_ps", bufs=3, space="PSUM") as mm_psp, \
             tc.tile_pool(name="tr_ps", bufs=1, space="PSUM") as tr_psp, \
             tc.tile_pool(name="oop_ps", bufs=3, space="PSUM") as oop_psp, \
             tc.tile_pool(name="dn_ps", bufs=1, space="PSUM") as dn_psp:
            qkT_sb = res_pool.tile([128, 3 * s], f32r)
            v_sb = res_pool.tile([128, s], at_st)
            ones_sb = res_pool.tile([128, 1], at_st)
            ident_sb = res_pool.tile([128, 128], f32)
            cmask_sb = res_pool.tile([128, 128], f32)
            oT_sb = res_pool.tile([128, HQ * s], op_st)
            woT_sb = res_pool.tile([128, HQ * h], op_st)
            eps_sb = res_pool.tile([128, 1], f32)
            rcT_sb = res_pool.tile([128, HQ * ST], f32)
            wall_sb = res_pool.tile([128, HT * 512], x_st)
            nc.vector.memset(eps_sb[:], EPS)
            nc.vector.memset(ones_sb[:], 1.0)
            nc.sync.dma_start(ident_sb[:], ident[:, :])
            nc.sync.dma_start(cmask_sb[:], cmask[:, :])

            # exp column buffers: ec[hd][i] holds exp(scoresT) for k-tile i,
            # current q-chunk. Written in panel p, consumed by attn@v(p).
            ecs = [[ec_pool.tile([128, 512], at_st, tag=f"ec{hd}_{i}",
                                 name=f"ec{hd}_{i}")
                    for i in range(ST)] for hd in range(HQ)]

            wq4 = 2

            def load_wall_chunk(c):
                nc.sync.dma_start(
                    wall_sb[:, c * 512:(c + wq4) * 512]
                    .rearrange("p (t n) -> p t n", n=512),
                    wall[c * 128:(c + wq4) * 128, :]
                    .rearrange("(t p) n -> p t n", p=128))

            def emit_proj_mm(m, xp, mi, ssq_p):
                """projection matmuls + squares + raw copies for s-tile m;
                returns SBUF tiles for the deferred rope."""
                qkv_ps = mm_psp.tile([128, 512], f32, tag="mm", name="qkv_ps")
                for t in range(HT):
                    nc.tensor.matmul(
                        qkv_ps[:],
                        lhsT=xp[:, t * 512 + mi * 128:t * 512 + mi * 128 + 128],
                        rhs=wall_sb[:, t * 512:(t + 1) * 512],
                        start=(t == 0), stop=(t == HT - 1))

                cs = csn_pool.tile([128, 512], f32, tag="cs", name="cs",
                                   bufs=6)
                nc.sync.dma_start(cs[:], csn[m * 128:(m + 1) * 128, :])

                sq = tmp_pool.tile([128, 128], f32, tag="sq", name="sq")
                qraw = tmp_pool.tile([128, 384], f32, tag="qraw",
                                     name="qraw", bufs=5)
                rot = tmp_pool.tile([128, 384], f32, tag="rot", name="rot",
                                    bufs=5)
                for g in range(3):
                    nc.scalar.activation(
                        sq[:], qkv_ps[:, g * 128:(g + 1) * 128],
                        Act.Square, accum_out=ssq_p[:, mi * 3 + g:
                                                    mi * 3 + g + 1])
                nc.scalar.copy(qraw[:], qkv_ps[:, 0:384])
                src_h = qkv_ps[:, 0:384].rearrange(
                    "p (g two e) -> p g two e", two=2, e=64)
                rot_h = rot.rearrange("p (g two e) -> p g two e", two=2, e=64)
                nc.vector.tensor_copy(rot_h[:, :, 0, :], src_h[:, :, 1, :])
                nc.vector.tensor_copy(rot_h[:, :, 1, :], src_h[:, :, 0, :])
                nc.scalar.copy(v_sb[:, m * 128:(m + 1) * 128],
                               qkv_ps[:, 384:512])
                return qraw, rot, cs

            def emit_rsqrt(ssq_p):
                """rs = 1/sqrt(ssq/D + eps) via Newton (no ACT table swap).
                Inputs are unit-variance so v is within [0.5, 2]; five
                iterations from y0=1 converge below fp32 epsilon."""
                w = 12
                v = tmp_pool.tile([128, w], f32, tag="nv", name="v", bufs=2)
                y = tmp_pool.tile([128, w], f32, tag="ny", name="y", bufs=2)
                t1 = tmp_pool.tile([128, w], f32, tag="nt1", name="t1",
                                   bufs=2)
                r = tmp_pool.tile([128, w], f32, tag="nr", name="r", bufs=2)
                nc.vector.tensor_scalar(v[:], ssq_p[:, 0:w], 1.0 / D, EPS,
                                        op0=Alu.mult, op1=Alu.add)
                nc.vector.reciprocal(r[:], v[:])
                # rsqrt(r) by Newton; r <= ~1.3 so the linear seed always
                # converges. rs = v^-1/2 = r * rsqrt(r).
                nc.vector.tensor_scalar(y[:], r[:], -0.5, 1.5,
                                        op0=Alu.mult, op1=Alu.add)
                for _ in range(4):
                    nc.vector.tensor_mul(t1[:], y[:], y[:])
                    nc.vector.tensor_mul(t1[:], t1[:], r[:])
                    nc.vector.tensor_scalar(t1[:], t1[:], -0.5, 1.5,
                                            op0=Alu.mult, op1=Alu.add)
                    nc.vector.tensor_mul(y[:], y[:], t1[:])
                nc.vector.tensor_mul(y[:], y[:], r[:])
                return y

            def emit_rope(m, mi, qraw, rot, cs, rs):
                """deferred rope + transposes for s-tile m"""
                qn = tmp_pool.tile([128, 384], f32, tag="qn", name="qn")
                t2 = tmp_pool.tile([128, 384], f32, tag="t2", name="t2")
                for g in range(3):
                    c0 = 0 if g < 2 else 128
                    nc.vector.scalar_tensor_tensor(
                        out=qn[:, g * 128:(g + 1) * 128],
                        in0=qraw[:, g * 128:(g + 1) * 128],
                        scalar=rs[:, mi * 3 + g:mi * 3 + g + 1],
                        in1=cs[:, c0:c0 + 128],
                        op0=Alu.mult, op1=Alu.mult)
                    nc.vector.scalar_tensor_tensor(
                        out=t2[:, g * 128:(g + 1) * 128],
                        in0=rot[:, g * 128:(g + 1) * 128],
                        scalar=rs[:, mi * 3 + g:mi * 3 + g + 1],
                        in1=cs[:, 256 + c0:256 + c0 + 128],
                        op0=Alu.mult, op1=Alu.mult)
                nc.vector.tensor_add(qn[:, 0:384], qn[:, 0:384],
                                     t2[:, 0:384])
                trp = tr_psp.tile([128, 384], f32, tag="trp", name="trp")
                for g in range(3):
                    nc.tensor.transpose(
                        trp[:, g * 128:(g + 1) * 128],
                        qn[:, g * 128:(g + 1) * 128], ident_sb[:])
                dst = qkT_sb.rearrange("p (g x) -> p g x", g=3)[
                    :, :, m * 128:m * 128 + 128]
                nc.scalar.copy(dst, trp.rearrange("p (g x) -> p g x", g=3))

            def emit_scores(i, pc):
                """scoresT chunk for k-tile i, q-chunk pc, both heads"""
                dlo = 128 * i
                seg = pc * 512
                lo = max(seg, dlo)
                for hd in range(HQ):
                    ec = ecs[hd][i]
                    if lo > seg:
                        nc.vector.memset(ec[:, 0:lo - seg], 0.0)
                    sc = mm_psp.tile([128, 512], f32, tag="mm", name="sc")
                    nc.tensor.matmul(
                        sc[:, lo - seg:512],
                        lhsT=qkT_sb[:, 2 * s + i * 128:2 * s + (i + 1) * 128],
                        rhs=qkT_sb[:, hd * s + lo:hd * s + seg + 512],
                        start=True, stop=True)
                    if seg <= dlo:
                        nc.vector.tensor_add(
                            sc[:, dlo - seg:dlo - seg + 128],
                            sc[:, dlo - seg:dlo - seg + 128],
                            cmask_sb[:])
                    nc.scalar.activation(
                        ec[:, lo - seg:512], sc[:, lo - seg:512],
                        Act.Exp, scale=inv_sqrt_d)

            def start_attnv(pc):
                dn_ps = dn_psp.tile([33, 512], f32, tag="dn", name="dn_ps")
                o_ps0 = oop_psp.tile([128, 512], f32, tag="oop", name="o_ps0")
                o_ps1 = oop_psp.tile([128, 512], f32, tag="oop", name="o_ps1")
                return (dn_ps, [o_ps0, o_ps1])

            def attnv_mm(i, pc, st):
                imax = min(ST, 4 * pc + 4)
                dn_ps, o_pss = st
                for hd in range(HQ):
                    nc.tensor.matmul(o_pss[hd][:],
                                     lhsT=v_sb[:, i * 128:(i + 1) * 128],
                                     rhs=ecs[hd][i][:],
                                     start=(i == 0), stop=(i == imax - 1))
                    nc.tensor.matmul(dn_ps[hd * 32:hd * 32 + 1, :],
                                     lhsT=ones_sb[:],
                                     rhs=ecs[hd][i][:],
                                     start=(i == 0), stop=(i == imax - 1))

            def finish_attnv(pc, st):
                dn_ps, o_pss = st
                for hd in range(HQ):
                    dst = oT_sb[:, hd * s + pc * 512:hd * s + (pc + 1) * 512]
                    if hd == 0:
                        nc.scalar.copy(dst, o_pss[hd][:])
                    else:
                        nc.vector.tensor_copy(dst, o_pss[hd][:])
                dn_sb = dnsb_pool.tile([33, 512], f32, tag="dn_sb",
                                       name="dn_sb")
                nc.scalar.copy(dn_sb[:], dn_ps[:])
                dnt = tr_psp.tile([128, HQ * 4], f32, tag="trp", name="dnt")
                for hd in range(HQ):
                    for c in range(4):
                        nc.tensor.transpose(
                            dnt[:, hd * 4 + c:hd * 4 + c + 1],
                            dn_sb[hd * 32:hd * 32 + 1,
                                  c * 128:(c + 1) * 128],
                            ident_sb[hd * 32:hd * 32 + 1,
                                     hd * 32:hd * 32 + 1])
                for hd in range(HQ):
                    nc.vector.reciprocal(
                        rcT_sb[:, hd * ST + pc * 4:hd * ST + pc * 4 + 4],
                        dnt[:, hd * 4:hd * 4 + 4])

            def emit_oproj(pc):
                """output projection for the s-tiles of q-chunk pc"""
                for m4 in range(4):
                    m = pc * 4 + m4
                    for n in range(NOC):
                        ops = []
                        for hd in range(HQ):
                            op_ps = oop_psp.tile([128, 512], f32, tag="oop",
                                                 name="op_ps")
                            ops.append(op_ps)
                            nc.tensor.matmul(
                                op_ps[:],
                                lhsT=oT_sb[:, hd * s + m * 128:
                                           hd * s + m * 128 + 128],
                                rhs=woT_sb[:, hd * h + n * 512:
                                           hd * h + (n + 1) * 512],
                                start=True, stop=True)
                        ob = out_pool.tile([128, 512], out_dt, tag="ob",
                                           name="ob")
                        r0 = rcT_sb[:, 0 * ST + m:0 * ST + m + 1]
                        r1 = rcT_sb[:, 1 * ST + m:1 * ST + m + 1]
                        if (m * NOC + n) % 2 == 0:
                            nc.scalar.activation(ob[:], ops[0][:], Act.Copy,
                                                 scale=r0)
                        else:
                            nc.vector.tensor_scalar_mul(ob[:], ops[0][:], r0)
                        nc.vector.scalar_tensor_tensor(
                            out=ob[:], in0=ops[1][:], scalar=r1,
                            in1=ob[:], op0=Alu.mult, op1=Alu.add)
                        nc.sync.dma_start(
                            out[m * 128:(m + 1) * 128, n * 512:(n + 1) * 512],
                            ob[:])

            # ------- pipelined panel loop: proj(p) | scores+attnv(p-1)
            # | o-proj(p-2) ----------------------------------------------
            for pn in range(NP + 1):
                if pn < NP:
                    xp = xp_pool.tile([128, HT * 512], x_st, tag="xp",
                                      name="xp")
                    xh = 2 if pn == 0 else max(1, HT // 4)
                    for c in range(0, HT, xh):
                        if pn == 0:
                            for cw in range(c, c + xh, wq4):
                                load_wall_chunk(cw)
                        nc.sync.dma_start(
                            xp[:, c * 512:(c + xh) * 512]
                            .rearrange("p (t e) -> p t e", e=512),
                            xT[c * 128:(c + xh) * 128,
                               pn * 512:(pn + 1) * 512]
                            .rearrange("(t p) e -> p t e", p=128))
                    if pn == min(1, NP - 1):
                        for g in range(HQ):
                            nc.sync.dma_start(
                                woT_sb[:, g * h:(g + 1) * h],
                                woT[g * 128:(g + 1) * 128, :]
                                .rearrange("(g p) o -> p (g o)", g=1))
                sc_list = (list(range(min(ST, 4 * (pn - 1) + 4)))
                           if pn >= 1 else [])
                parts = []
                if pn < NP:
                    ssq_p = tmp_pool.tile([128, 12], f32, tag="ssq_p",
                                          name="ssq_p", bufs=2)
                for mi in range(4):
                    if pn < NP:
                        parts.append(emit_proj_mm(pn * 4 + mi, xp, mi, ssq_p))
                    for i in sc_list[mi::4]:
                        emit_scores(i, pn - 1)
                if pn >= 1:
                    st = start_attnv(pn - 1)
                    for i in sc_list:
                        attnv_mm(i, pn - 1, st)
                    finish_attnv(pn - 1, st)
                if pn < NP:
                    rs = emit_rsqrt(ssq_p)
                    for mi in range(4):
                        qraw, rot, cs = parts[mi]
                        emit_rope(pn * 4 + mi, mi, qraw, rot, cs, rs)
                if pn >= 2:
                    emit_oproj(pn - 2)
            emit_oproj(NP - 1)

    nc.finalize()
    return nc


def host_inputs(x, sin, cos, wq, wk, wv, wo, q_norm_w, k_norm_w,
                dt_x=None, dt_oproj=None):
    """Build the 8 per-core input maps from full inputs."""
    dt_x = dt_x or CFG[0]
    dt_oproj = dt_oproj or CFG[2]
    import ml_dtypes
    bfnp = ml_dtypes.bfloat16

    def cast_x(a):
        if dt_x == "bf16":
            return np.ascontiguousarray(a).astype(bfnp)
        return np.ascontiguousarray(a, dtype=np.float32)

    def cast_o(a):
        if dt_oproj == "bf16":
            return np.ascontiguousarray(a).astype(bfnp)
        return np.ascontiguousarray(a, dtype=np.float32)

    xT = cast_x(x[0].T)

    sin2 = sin[0, 0]  # [S, D]
    cos2 = cos[0, 0]
    sign = np.where(np.arange(D) < 64, -1.0, 1.0).astype(np.float32)
    perm = (np.arange(D) + 64) % D
    cos_q = (cos2 * q_norm_w[None, :]).astype(np.float32)
    cos_k = (cos2 * k_norm_w[None, :]).astype(np.float32)
    sin_q = (sin2 * sign[None, :] * q_norm_w[None, perm]).astype(np.float32)
    sin_k = (sin2 * sign[None, :] * k_norm_w[None, perm]).astype(np.float32)
    csn = np.ascontiguousarray(
        np.concatenate([cos_q, cos_k, sin_q, sin_k], axis=1), dtype=np.float32)

    kq = np.arange(128)
    cmask = np.where(kq[None, :] >= kq[:, None], 0.0, NEG).astype(np.float32)
    ident = np.eye(128, dtype=np.float32)

    in_maps = []
    for d in range(N_CORES):
        kv = d // 2
        wall = cast_x(np.concatenate(
            [wq[256 * d:256 * (d + 1)],
             wk[128 * kv:128 * (kv + 1)],
             wv[128 * kv:128 * (kv + 1)]], axis=0).T)
        woTd = cast_o(wo[:, 256 * d:256 * (d + 1)].T)
        in_maps.append({"xT": xT, "wall": wall, "woT": woTd, "csn": csn,
                        "cmask": cmask, "ident": ident})
    return in_maps


_NC_CACHE = {}


def kernel(x, sin, cos, mask, wq, wk, wv, wo, q_norm_w, k_norm_w,
           _trace=False):
    from concourse.bass_utils import run_bass_kernel_spmd

    key = CFG
    if key not in _NC_CACHE:
        _NC_CACHE[key] = build_nc(s=S, h=H)
    nc = _NC_CACHE[key]

    in_maps = host_inputs(np.asarray(x), np.asarray(sin), np.asarray(cos),
                          np.asarray(wq), np.asarray(wk), np.asarray(wv),
                          np.asarray(wo), np.asarray(q_norm_w),
                          np.asarray(k_norm_w))
    res = run_bass_kernel_spmd(nc, in_maps, core_ids=list(range(N_CORES)),
                               trace=_trace)
    acc = np.zeros((S, H), dtype=np.float32)
    for d in range(N_CORES):
        acc += np.asarray(res.results[d]["out"], dtype=np.float32)
    outv = acc.reshape(1, S, H)
    if _trace:
        return outv, res
    return outv


# revision 32
# speedup vs baseline: 1.1868x; 1.0676x over previous
"""Trainium2 Bass kernel for GQA attention (nn_Attention_36739150250506).

B=1, S=2048, H=2048, NH=16 q-heads, NKV=4 kv-heads, D=128.
Sharding: 2 q-heads + their (shared) kv-head per core; o-proj partial sums
reduced on host.

Single pipelined loop over 512-wide sequence panels keeps the tensor engine
dense (HAM stays warm). Per panel p (all matmuls contract over partitions):
  1. qkv = xT.T @ [wq0|wq1|wk|wv] for the panel's four 128-row s-tiles;
     RMSNorm+RoPE in [s,d] layout (norm weights folded into host cos/sin
     tables); PE-transpose q/k to [d,s]; v kept [s,d] bf16.
  2. scoresT[k, q-chunk p] = kT_i.T @ qT for all causal k-tiles i; diag
     mask add; exp on ACT (scale=1/sqrt(D)) into per-column bf16 buffers.
  3. attn@v for q-chunk p (v.T @ expT, unnormalized) + denominator via
     ones-matmul; denominator rows PE-transposed to per-partition layout,
     reciprocal into a [s-tile] table.
  4. o-proj for q-chunk p-1: per-head matmuls, evict applies the
     reciprocal as a per-partition scale, partial written to HBM in bf16.
"""

import math

import numpy as np

S = 2048
H = 2048
D = 128
NH = 16
NKV = 4
HQ = 2  # q heads per core
N_CORES = 8
EPS = 1.1920928955078125e-07
NEG = -1e30
CFG = ("bf16", "bf16", "bf16")  # (dt_x, dt_attn, dt_oproj)
OUT_BF16 = True


def build_nc(s=S, h=H, dt_x=None, dt_attn=None, dt_oproj=None):
    dt_x = dt_x or CFG[0]
    dt_attn = dt_attn or CFG[1]
    dt_oproj = dt_oproj or CFG[2]
    import concourse.bacc as bacc
    import concourse.mybir as mybir
    import concourse.tile as tile

    f32 = mybir.dt.float32
    f32r = mybir.dt.float32r
    bf16 = mybir.dt.bfloat16
    Act = mybir.ActivationFunctionType
    Alu = mybir.AluOpType

    def dtt(name):
        return {"f32": f32, "f32r": f32r, "bf16": bf16}[name]

    x_st = dtt(dt_x)
    at_st = dtt(dt_attn)
    op_st = dtt(dt_oproj)

    ST = s // 128   # 128-row s-tiles
    HT = h // 128   # 128-row h-tiles (contraction tiles)
    NOC = h // 512  # 512-wide outdim chunks
    NP = s // 512   # 512-wide panels / q-chunks

    nc = bacc.Bacc("TRN2", target_bir_lowering=False, debug=False,
                   num_devices=N_CORES)

    xT = nc.dram_tensor("xT", [h, s], x_st, kind="ExternalInput")
    wall = nc.dram_tensor("wall", [h, 512], x_st, kind="ExternalInput")
    woT = nc.dram_tensor("woT", [HQ * 128, h], op_st, kind="ExternalInput")
    csn = nc.dram_tensor("csn", [s, 512], f32, kind="ExternalInput")
    cmask = nc.dram_tensor("cmask", [128, 128], f32, kind="ExternalInput")
    ident = nc.dram_tensor("ident", [128, 128], f32, kind="ExternalInput")
    out_dt = bf16 if OUT_BF16 else f32
    out = nc.dram_tensor("out", [s, h], out_dt, kind="ExternalOutput")

    inv_sqrt_d = 1.0 / math.sqrt(D)

    with tile.TileContext(nc) as tc:
        with tc.tile_pool(name="resident", bufs=1) as res_pool, \
             tc.tile_pool(name="ecol", bufs=1) as ec_pool, \
             tc.tile_pool(name="xpanel", bufs=2) as xp_pool, \
             tc.tile_pool(name="csn_p", bufs=3) as csn_pool, \
             tc.tile_pool(name="ph1tmp", bufs=2) as tmp_pool, \
             tc.tile_pool(name="dn_sb_p", bufs=2) as dnsb_pool, \
             tc.tile_pool(name="out_sb", bufs=4) as out_pool, \
             tc.tile_pool(name="mm_ps", bufs=3, space="PSUM") as mm_psp, \
             tc.tile_pool(name="tr_ps", bufs=1, space="PSUM") as tr_psp, \
             tc.tile_pool(name="oop_ps", bufs=3, space="PSUM") as oop_psp, \
             tc.tile_pool(name="dn_ps", bufs=1, space="PSUM") as dn_psp:
            qkT_sb = res_pool.tile([128, 3 * s], f32r)
            v_sb = res_pool.tile([128, s], at_st)
            ones_sb = res_pool.tile([128, 1], at_st)
            ident_sb = res_pool.tile([128, 128], f32)
            cmask_sb = res_pool.tile([128, 128], f32)
            oT_sb = res_pool.tile([128, HQ * s], op_st)
            woT_sb = res_pool.tile([128, HQ * h], op_st)
            eps_sb = res_pool.tile([128, 1], f32)
            rcT_sb = res_pool.tile([128, HQ * ST], f32)
            wall_sb = res_pool.tile([128, HT * 512], x_st)
            nc.vector.memset(eps_sb[:], EPS)
            nc.vector.memset(ones_sb[:], 1.0)
            nc.sync.dma_start(ident_sb[:], ident[:, :])
            nc.sync.dma_start(cmask_sb[:], cmask[:, :])

            # exp column buffers: ec[hd][i] holds exp(scoresT) for k-tile i,
            # current q-chunk. Written in panel p, consumed by attn@v(p).
            ecs = [[ec_pool.tile([128, 512], at_st, tag=f"ec{hd}_{i}",
                                 name=f"ec{hd}_{i}")
                    for i in range(ST)] for hd in range(HQ)]

            wq4 = 2

            def load_wall_chunk(c):
                nc.sync.dma_start(
                    wall_sb[:, c * 512:(c + wq4) * 512]
                    .rearrange("p (t n) -> p t n", n=512),
                    wall[c * 128:(c + wq4) * 128, :]
                    .rearrange("(t p) n -> p t n", p=128))

            def emit_proj_mm(m, xp, mi, ssq_p):
                """projection matmuls + squares + raw copies for s-tile m;
                returns SBUF tiles for the deferred rope."""
                qkv_ps = mm_psp.tile([128, 512], f32, tag="mm", name="qkv_ps")
                for t in range(HT):
                    nc.tensor.matmul(
                        qkv_ps[:],
                        lhsT=xp[:, t * 512 + mi * 128:t * 512 + mi * 128 + 128],
                        rhs=wall_sb[:, t * 512:(t + 1) * 512],
                        start=(t == 0), stop=(t == HT - 1))

                cs = csn_pool.tile([128, 512], f32, tag="cs", name="cs",
                                   bufs=6)
                nc.sync.dma_start(cs[:], csn[m * 128:(m + 1) * 128, :])

                sq = tmp_pool.tile([128, 128], f32, tag="sq", name="sq")
                qraw = tmp_pool.tile([128, 384], f32, tag="qraw",
                                     name="qraw", bufs=5)
                rot = tmp_pool.tile([128, 384], f32, tag="rot", name="rot",
                                    bufs=5)
                for g in range(3):
                    nc.scalar.activation(
                        sq[:], qkv_ps[:, g * 128:(g + 1) * 128],
                        Act.Square, accum_out=ssq_p[:, mi * 3 + g:
                                                    mi * 3 + g + 1])
                nc.scalar.copy(qraw[:], qkv_ps[:, 0:384])
                src_h = qkv_ps[:, 0:384].rearrange(
                    "p (g two e) -> p g two e", two=2, e=64)
                rot_h = rot.rearrange("p (g two e) -> p g two e", two=2, e=64)
                nc.vector.tensor_copy(rot_h[:, :, 0, :], src_h[:, :, 1, :])
                nc.vector.tensor_copy(rot_h[:, :, 1, :], src_h[:, :, 0, :])
                nc.scalar.copy(v_sb[:, m * 128:(m + 1) * 128],
                               qkv_ps[:, 384:512])
                return qraw, rot, cs

            def emit_rsqrt(ssq_p, c0=0, w=12):
                """rs = 1/sqrt(ssq/D + eps) via Newton (no ACT table swap)."""
                v = tmp_pool.tile([128, w], f32, tag="nv", name="v", bufs=2)
                y = tmp_pool.tile([128, w], f32, tag="ny", name="y", bufs=2)
                t1 = tmp_pool.tile([128, w], f32, tag="nt1", name="t1",
                                   bufs=2)
                r = tmp_pool.tile([128, w], f32, tag="nr", name="r", bufs=2)
                nc.vector.tensor_scalar(v[:], ssq_p[:, c0:c0 + w], 1.0 / D, EPS,
                                        op0=Alu.mult, op1=Alu.add)
                nc.vector.reciprocal(r[:], v[:])
                # rsqrt(r) by Newton; r <= ~1.3 so the linear seed always
                # converges. rs = v^-1/2 = r * rsqrt(r).
                nc.vector.tensor_scalar(y[:], r[:], -0.5, 1.5,
                                        op0=Alu.mult, op1=Alu.add)
                for _ in range(4):
                    nc.vector.tensor_mul(t1[:], y[:], y[:])
                    nc.vector.tensor_mul(t1[:], t1[:], r[:])
                    nc.vector.tensor_scalar(t1[:], t1[:], -0.5, 1.5,
                                            op0=Alu.mult, op1=Alu.add)
                    nc.vector.tensor_mul(y[:], y[:], t1[:])
                nc.vector.tensor_mul(y[:], y[:], r[:])
                return y

            def emit_rope(m, mi, qraw, rot, cs, rs, rbase):
                """deferred rope + transposes for s-tile m"""
                qn = tmp_pool.tile([128, 384], f32, tag="qn", name="qn")
                t2 = tmp_pool.tile([128, 384], f32, tag="t2", name="t2")
                for g in range(3):
                    c0 = 0 if g < 2 else 128
                    nc.vector.scalar_tensor_tensor(
                        out=qn[:, g * 128:(g + 1) * 128],
                        in0=qraw[:, g * 128:(g + 1) * 128],
                        scalar=rs[:, rbase + g:rbase + g + 1],
                        in1=cs[:, c0:c0 + 128],
                        op0=Alu.mult, op1=Alu.mult)
                    nc.vector.scalar_tensor_tensor(
                        out=t2[:, g * 128:(g + 1) * 128],
                        in0=rot[:, g * 128:(g + 1) * 128],
                        scalar=rs[:, rbase + g:rbase + g + 1],
                        in1=cs[:, 256 + c0:256 + c0 + 128],
                        op0=Alu.mult, op1=Alu.mult)
                nc.vector.tensor_add(qn[:, 0:384], qn[:, 0:384],
                                     t2[:, 0:384])
                trp = tr_psp.tile([128, 384], f32, tag="trp", name="trp")
                for g in range(3):
                    nc.tensor.transpose(
                        trp[:, g * 128:(g + 1) * 128],
                        qn[:, g * 128:(g + 1) * 128], ident_sb[:])
                dst = qkT_sb.rearrange("p (g x) -> p g x", g=3)[
                    :, :, m * 128:m * 128 + 128]
                nc.scalar.copy(dst, trp.rearrange("p (g x) -> p g x", g=3))

            def emit_scores(i, pc):
                """scoresT chunk for k-tile i, q-chunk pc, both heads"""
                dlo = 128 * i
                seg = pc * 512
                lo = max(seg, dlo)
                for hd in range(HQ):
                    ec = ecs[hd][i]
                    if lo > seg:
                        nc.vector.memset(ec[:, 0:lo - seg], 0.0)
                    sc = mm_psp.tile([128, 512], f32, tag="mm", name="sc")
                    nc.tensor.matmul(
                        sc[:, lo - seg:512],
                        lhsT=qkT_sb[:, 2 * s + i * 128:2 * s + (i + 1) * 128],
                        rhs=qkT_sb[:, hd * s + lo:hd * s + seg + 512],
                        start=True, stop=True)
                    if seg <= dlo:
                        nc.vector.tensor_add(
                            sc[:, dlo - seg:dlo - seg + 128],
                            sc[:, dlo - seg:dlo - seg + 128],
                            cmask_sb[:])
                    nc.scalar.activation(
                        ec[:, lo - seg:512], sc[:, lo - seg:512],
                        Act.Exp, scale=inv_sqrt_d)

            def start_attnv(pc):
                dn_ps = dn_psp.tile([33, 512], f32, tag="dn", name="dn_ps")
                o_ps0 = oop_psp.tile([128, 512], f32, tag="oop", name="o_ps0")
                o_ps1 = oop_psp.tile([128, 512], f32, tag="oop", name="o_ps1")
                return (dn_ps, [o_ps0, o_ps1])

            def attnv_mm(i, pc, st):
                imax = min(ST, 4 * pc + 4)
                dn_ps, o_pss = st
                for hd in range(HQ):
                    nc.tensor.matmul(o_pss[hd][:],
                                     lhsT=v_sb[:, i * 128:(i + 1) * 128],
                                     rhs=ecs[hd][i][:],
                                     start=(i == 0), stop=(i == imax - 1))
                    nc.tensor.matmul(dn_ps[hd * 32:hd * 32 + 1, :],
                                     lhsT=ones_sb[:],
                                     rhs=ecs[hd][i][:],
                                     start=(i == 0), stop=(i == imax - 1))

            def finish_attnv(pc, st):
                dn_ps, o_pss = st
                for hd in range(HQ):
                    dst = oT_sb[:, hd * s + pc * 512:hd * s + (pc + 1) * 512]
                    if hd == 0:
                        nc.scalar.copy(dst, o_pss[hd][:])
                    else:
                        nc.vector.tensor_copy(dst, o_pss[hd][:])
                dn_sb = dnsb_pool.tile([33, 512], f32, tag="dn_sb",
                                       name="dn_sb")
                nc.scalar.copy(dn_sb[:], dn_ps[:])
                dnt = tr_psp.tile([128, HQ * 4], f32, tag="trp", name="dnt")
                for hd in range(HQ):
                    for c in range(4):
                        nc.tensor.transpose(
                            dnt[:, hd * 4 + c:hd * 4 + c + 1],
                            dn_sb[hd * 32:hd * 32 + 1,
                                  c * 128:(c + 1) * 128],
                            ident_sb[hd * 32:hd * 32 + 1,
                                     hd * 32:hd * 32 + 1])
                for hd in range(HQ):
                    nc.vector.reciprocal(
                        rcT_sb[:, hd * ST + pc * 4:hd * ST + pc * 4 + 4],
                        dnt[:, hd * 4:hd * 4 + 4])

            def emit_oproj(pc):
                """output projection for the s-tiles of q-chunk pc"""
                for m4 in range(4):
                    m = pc * 4 + m4
                    for n in range(NOC):
                        ops = []
                        for hd in range(HQ):
                            op_ps = oop_psp.tile([128, 512], f32, tag="oop",
                                                 name="op_ps")
                            ops.append(op_ps)
                            nc.tensor.matmul(
                                op_ps[:],
                                lhsT=oT_sb[:, hd * s + m * 128:
                                           hd * s + m * 128 + 128],
                                rhs=woT_sb[:, hd * h + n * 512:
                                           hd * h + (n + 1) * 512],
                                start=True, stop=True)
                        ob = out_pool.tile([128, 512], out_dt, tag="ob",
                                           name="ob")
                        r0 = rcT_sb[:, 0 * ST + m:0 * ST + m + 1]
                        r1 = rcT_sb[:, 1 * ST + m:1 * ST + m + 1]
                        if (m * NOC + n) % 2 == 0:
                            nc.scalar.activation(ob[:], ops[0][:], Act.Copy,
                                                 scale=r0)
                        else:
                            nc.vector.tensor_scalar_mul(ob[:], ops[0][:], r0)
                        nc.vector.scalar_tensor_tensor(
                            out=ob[:], in0=ops[1][:], scalar=r1,
                            in1=ob[:], op0=Alu.mult, op1=Alu.add)
                        nc.sync.dma_start(
                            out[m * 128:(m + 1) * 128, n * 512:(n + 1) * 512],
                            ob[:])

            # ------- pipelined panel loop: proj(p) | scores+attnv(p-1)
            # | o-proj(p-2) ----------------------------------------------
            for pn in range(NP + 1):
                if pn < NP:
                    xp = xp_pool.tile([128, HT * 512], x_st, tag="xp",
                                      name="xp")
                    xh = 2 if pn == 0 else max(1, HT // 4)
                    for c in range(0, HT, xh):
                        if pn == 0:
                            for cw in range(c, c + xh, wq4):
                                load_wall_chunk(cw)
                        nc.sync.dma_start(
                            xp[:, c * 512:(c + xh) * 512]
                            .rearrange("p (t e) -> p t e", e=512),
                            xT[c * 128:(c + xh) * 128,
                               pn * 512:(pn + 1) * 512]
                            .rearrange("(t p) e -> p t e", p=128))
                    if pn == min(1, NP - 1):
                        for g in range(HQ):
                            nc.sync.dma_start(
                                woT_sb[:, g * h:(g + 1) * h],
                                woT[g * 128:(g + 1) * 128, :]
                                .rearrange("(g p) o -> p (g o)", g=1))
                sc_list = (list(range(min(ST, 4 * (pn - 1) + 4)))
                           if pn >= 1 else [])
                parts = []
                if pn < NP:
                    ssq_p = tmp_pool.tile([128, 12], f32, tag="ssq_p",
                                          name="ssq_p", bufs=2)
                for mi in range(4):
                    if pn < NP:
                        parts.append(emit_proj_mm(pn * 4 + mi, xp, mi, ssq_p))
                    for i in sc_list[mi::4]:
                        emit_scores(i, pn - 1)
                    if pn < NP and mi in (1, 3):
                        rs = emit_rsqrt(ssq_p, c0=(mi - 1) * 3, w=6)
                        for mj in (mi - 1, mi):
                            qraw, rot, cs = parts[mj]
                            emit_rope(pn * 4 + mj, mj, qraw, rot, cs, rs,
                                      (mj % 2) * 3)
                if pn >= 1:
                    st = start_attnv(pn - 1)
                    for i in sc_list:
                        attnv_mm(i, pn - 1, st)
                    finish_attnv(pn - 1, st)
                if pn >= 2:
                    emit_oproj(pn - 2)
            emit_oproj(NP - 1)

    nc.finalize()
    return nc


def host_inputs(x, sin, cos, wq, wk, wv, wo, q_norm_w, k_norm_w,
                dt_x=None, dt_oproj=None):
    """Build the 8 per-core input maps from full inputs."""
    dt_x = dt_x or CFG[0]
    dt_oproj = dt_oproj or CFG[2]
    import ml_dtypes
    bfnp = ml_dtypes.bfloat16

    def cast_x(a):
        if dt_x == "bf16":
            return np.ascontiguousarray(a).astype(bfnp)
        return np.ascontiguousarray(a, dtype=np.float32)

    def cast_o(a):
        if dt_oproj == "bf16":
            return np.ascontiguousarray(a).astype(bfnp)
        return np.ascontiguousarray(a, dtype=np.float32)

    xT = cast_x(x[0].T)

    sin2 = sin[0, 0]  # [S, D]
    cos2 = cos[0, 0]
    sign = np.where(np.arange(D) < 64, -1.0, 1.0).astype(np.float32)
    perm = (np.arange(D) + 64) % D
    cos_q = (cos2 * q_norm_w[None, :]).astype(np.float32)
    cos_k = (cos2 * k_norm_w[None, :]).astype(np.float32)
    sin_q = (sin2 * sign[None, :] * q_norm_w[None, perm]).astype(np.float32)
    sin_k = (sin2 * sign[None, :] * k_norm_w[None, perm]).astype(np.float32)
    csn = np.ascontiguousarray(
        np.concatenate([cos_q, cos_k, sin_q, sin_k], axis=1), dtype=np.float32)

    kq = np.arange(128)
    cmask = np.where(kq[None, :] >= kq[:, None], 0.0, NEG).astype(np.float32)
    ident = np.eye(128, dtype=np.float32)

    in_maps = []
    for d in range(N_CORES):
        kv = d // 2
        wall = cast_x(np.concatenate(
            [wq[256 * d:256 * (d + 1)],
             wk[128 * kv:128 * (kv + 1)],
             wv[128 * kv:128 * (kv + 1)]], axis=0).T)
        woTd = cast_o(wo[:, 256 * d:256 * (d + 1)].T)
        in_maps.append({"xT": xT, "wall": wall, "woT": woTd, "csn": csn,
                        "cmask": cmask, "ident": ident})
    return in_maps


_NC_CACHE = {}


def kernel(x, sin, cos, mask, wq, wk, wv, wo, q_norm_w, k_norm_w,
           _trace=False):
    from concourse.bass_utils import run_bass_kernel_spmd

    key = CFG
    if key not in _NC_CACHE:
        _NC_CACHE[key] = build_nc(s=S, h=H)
    nc = _NC_CACHE[key]

    in_maps = host_inputs(np.asarray(x), np.asarray(sin), np.asarray(cos),
                          np.asarray(wq), np.asarray(wk), np.asarray(wv),
                          np.asarray(wo), np.asarray(q_norm_w),
                          np.asarray(k_norm_w))
    res = run_bass_kernel_spmd(nc, in_maps, core_ids=list(range(N_CORES)),
                               trace=_trace)
    acc = np.zeros((S, H), dtype=np.float32)
    for d in range(N_CORES):
        acc += np.asarray(res.results[d]["out"], dtype=np.float32)
    outv = acc.reshape(1, S, H)
    if _trace:
        return outv, res
    return outv
